# revision 1
# baseline (speedup 1.0000x reference)
"""Trainium2 Bass kernel for nn_BasicTransformerBlock_35304631173827.

Sharding: 8 cores = 4 samples x 2 sequence halves. Each core computes its
1024-token half of one sample fully locally (self-attention K/V recomputed
over the full 2048-token sample -> zero collectives). bf16 matmuls with
fp32 PSUM accumulation; LayerNorm stats, softmax and residuals in fp32.
Large intermediates (h1T, kT, x1, x2, y-accumulator) bounce through DRAM
to fit SBUF.
"""

import numpy as np
import ml_dtypes

BF16 = ml_dtypes.bfloat16

B, N, D = 4, 2048, 1024
J, CD = 256, 768
H, DH = 16, 64
INNER = 1024
FF = 4096
P = 128
KT = D // P            # 8
CKT = CD // P          # 6
TT_FULL = N // P       # 16
N_OWN = N // 2
TT_OWN = N_OWN // P    # 8
EPS = 1e-5

_CACHE = {}


def _build_program():
    import concourse.tile as tile
    from concourse import mybir, bacc
    from concourse.masks import make_identity
    from contextlib import ExitStack

    f32 = mybir.dt.float32
    bf16 = mybir.dt.bfloat16
    AF = mybir.ActivationFunctionType
    ALU = mybir.AluOpType

    nc = bacc.Bacc(None, target_bir_lowering=False)

    xf_d = nc.dram_tensor("xf", [TT_FULL, P, D], f32, kind="ExternalInput")
    tT_d = nc.dram_tensor("tT", [P, KT], bf16, kind="ExternalInput")
    nw_d = nc.dram_tensor("nw", [P, KT, 6 * D], bf16, kind="ExternalInput")
    nbc_d = nc.dram_tensor("nbc", [P, 48], f32, kind="ExternalInput")
    wq1_d = nc.dram_tensor("wq1", [P, KT, INNER], bf16, kind="ExternalInput")
    wk1_d = nc.dram_tensor("wk1", [P, KT, INNER], bf16, kind="ExternalInput")
    wv1_d = nc.dram_tensor("wv1", [P, KT, INNER], bf16, kind="ExternalInput")
    wo1_d = nc.dram_tensor("wo1", [P, KT, D], bf16, kind="ExternalInput")
    wq2_d = nc.dram_tensor("wq2", [P, KT, INNER], bf16, kind="ExternalInput")
    wk2_d = nc.dram_tensor("wk2", [P, CKT, INNER], bf16, kind="ExternalInput")
    wv2_d = nc.dram_tensor("wv2", [P, CKT, INNER], bf16, kind="ExternalInput")
    wo2_d = nc.dram_tensor("wo2", [P, KT, D], bf16, kind="ExternalInput")
    ctxT_d = nc.dram_tensor("ctxT", [P, CKT, J], bf16, kind="ExternalInput")
    bias3_d = nc.dram_tensor("bias3", [P, 3, D], f32, kind="ExternalInput")
    fb1_d = nc.dram_tensor("fb1c", [P, 64], f32, kind="ExternalInput")
    wf1_d = nc.dram_tensor("wf1", [P, KT, 2 * FF], bf16, kind="ExternalInput")
    wf2_d = nc.dram_tensor("wf2", [P, FF // P, D], bf16, kind="ExternalInput")
    y_d = nc.dram_tensor("y", [TT_OWN, P, D], f32, kind="ExternalOutput")

    with tile.TileContext(nc) as tc, ExitStack() as es:
        konst = es.enter_context(tc.tile_pool(name="konst", bufs=1))
        xpool = es.enter_context(tc.tile_pool(name="xpool", bufs=3))
        stats = es.enter_context(tc.tile_pool(name="stats", bufs=2))
        wres = es.enter_context(tc.tile_pool(name="wres", bufs=2))
        wsm = es.enter_context(tc.tile_pool(name="wsm", bufs=3))
        wmed = es.enter_context(tc.tile_pool(name="wmed", bufs=2))
        evict = es.enter_context(tc.tile_pool(name="evict", bufs=2))
        stg = es.enter_context(tc.tile_pool(name="stg", bufs=2))
        big = es.enter_context(tc.tile_pool(name="big", bufs=1))
        kthp = es.enter_context(tc.tile_pool(name="kthp", bufs=2))
        expp = es.enter_context(tc.tile_pool(name="expp", bufs=2))
        smk = es.enter_context(tc.tile_pool(name="smk", bufs=1))
        dramp = es.enter_context(tc.tile_pool(name="dramp", bufs=1, space="DRAM"))
        ps_a = es.enter_context(tc.tile_pool(name="ps_a", bufs=2, space="PSUM"))
        ps_sc = es.enter_context(tc.tile_pool(name="ps_sc", bufs=2, space="PSUM"))
        ps_av = es.enter_context(tc.tile_pool(name="ps_av", bufs=2, space="PSUM"))
        ps_tr = ps_av

        # ---------------- constants ----------------
        ident = konst.tile([P, P], bf16)
        make_identity(nc, ident)
        ones64 = konst.tile([1, 64], bf16)
        nc.vector.memset(ones64[:], 1.0)
        eps_t = konst.tile([P, 1], f32)
        nc.vector.memset(eps_t[:], EPS)
        tT_sb = konst.tile([P, KT], bf16)
        nc.sync.dma_start(tT_sb[:], tT_d[:])
        nbc_sb = konst.tile([P, 48], f32)
        nc.sync.dma_start(nbc_sb[:], nbc_d[:])
        fb1_sb = konst.tile([P, 64], f32)
        nc.sync.dma_start(fb1_sb[:], fb1_d[:])
        ctxT_sb = konst.tile([P, CKT, J], bf16)
        nc.sync.dma_start(ctxT_sb[:], ctxT_d[:])
        cols = konst.tile([P, 48], f32)

        # DRAM scratch (ExternalOutput so they double as debug dumps)
        h1T_dram = nc.dram_tensor("dbg_h1T", [P, KT, N], bf16, kind="ExternalOutput")
        kT_dram = nc.dram_tensor("dbg_kT", [KT, P, N], bf16, kind="ExternalOutput")
        x1_dram = nc.dram_tensor("dbg_x1", [TT_OWN, P, D], f32, kind="ExternalOutput")
        x2_dram = nc.dram_tensor("dbg_x2", [TT_OWN, P, D], f32, kind="ExternalOutput")
        dbg_cols = nc.dram_tensor("dbg_cols", [P, 48], f32, kind="ExternalOutput")
        dbg_q = nc.dram_tensor("dbg_q", [P, KT, N_OWN], bf16, kind="ExternalOutput")
        dbg_a1 = nc.dram_tensor("dbg_a1", [P, KT, N_OWN], bf16, kind="ExternalOutput")

        # ---------------- Phase 0: AdaLN embeddings (transposed: M=128,N=1) ----
        for c in range(48):
            nwt = wsm.tile([P, KT, P], bf16, tag="wstream")
            nc.sync.dma_start(nwt[:], nw_d[:, :, c * P:(c + 1) * P])
            ps = ps_a.tile([P, 512], f32, tag="psa")
            for kt in range(KT):
                nc.tensor.matmul(ps[:, 0:1], nwt[:, kt, :], tT_sb[:, kt:kt + 1],
                                 start=(kt == 0), stop=(kt == KT - 1))
            nc.vector.tensor_copy(cols[:, c:c + 1], ps[:, 0:1])
        nc.vector.tensor_add(cols[:], cols[:], nbc_sb[:])
        for n3 in range(3):
            nc.vector.tensor_scalar_add(cols[:, n3 * 16:n3 * 16 + 8],
                                        cols[:, n3 * 16:n3 * 16 + 8], 1.0)

        nc.sync.dma_start(dbg_cols[:], cols[:])

        def layernorm_tile(x_tile, tt, n3, dst_sb=None, dst_dram=None):
            """LayerNorm + AdaLN affine on (P, D) tile -> transposed chunks."""
            bst = stats.tile([P, 2, 6], f32, tag="bnst")
            for g in range(2):
                nc.vector.bn_stats(bst[:, g, :], x_tile[:, g * 512:(g + 1) * 512])
            mv = stats.tile([P, 4], f32, tag="mv")
            nc.vector.bn_aggr(mv[:, 0:2], bst[:])
            nc.scalar.activation(mv[:, 2:3], mv[:, 1:2], AF.Sqrt, bias=eps_t[:])
            nc.vector.reciprocal(mv[:, 2:3], mv[:, 2:3])
            nc.vector.tensor_tensor(mv[:, 3:4], mv[:, 0:1], mv[:, 2:3], ALU.mult)
            nc.vector.tensor_scalar_mul(mv[:, 3:4], mv[:, 3:4], -1.0)
            xn = evict.tile([P, D], bf16, tag="xn")
            nc.scalar.activation(xn[:], x_tile[:], AF.Identity,
                                 bias=mv[:, 3:4], scale=mv[:, 2:3])
            if dst_dram is not None:
                stage = stg.tile([P, KT, P], bf16, tag="stage", name="stage")
            else:
                stage = None
            for c in range(KT):
                pt = ps_tr.tile([P, P], bf16, tag="psav")
                nc.tensor.transpose(pt[:], xn[:, c * P:(c + 1) * P], ident[:])
                out_ap = (stage[:, c, :] if dst_dram is not None
                          else dst_sb[:, c, tt * P:(tt + 1) * P])
                nc.vector.tensor_scalar(
                    out_ap, pt[:],
                    cols[:, n3 * 16 + c:n3 * 16 + c + 1],
                    cols[:, n3 * 16 + 8 + c:n3 * 16 + 8 + c + 1],
                    ALU.mult, ALU.add)
            if dst_dram is not None:
                nc.sync.dma_start(dst_dram[:, :, tt * P:(tt + 1) * P], stage[:])

        # ---------------- Phase 1: LN1 (full sample) -> h1T_dram ----------------
        for tt in range(TT_FULL):
            xt = xpool.tile([P, D], f32, tag="x")
            nc.sync.dma_start(xt[:], xf_d[tt])
            layernorm_tile(xt, tt, 0, dst_dram=h1T_dram)

        # ---------------- Phase 2: QKV projections ----------------
        qT = big.tile([P, KT, N_OWN], bf16, tag="qT")
        SC = DH ** -0.5

        def qk_proj(w_dram, n_tok, out_sb, out_dram, scale):
            w_sb = wres.tile([P, KT, INNER], bf16, tag="wbig")
            nc.sync.dma_start(w_sb[:], w_dram[:])
            for qc in range(n_tok // 256):
                hch = wmed.tile([P, KT, 256], bf16, tag="med4")
                nc.sync.dma_start(hch[:], h1T_dram[:, :, qc * 256:(qc + 1) * 256])
                for m in range(KT):
                    ps = ps_a.tile([P, 512], f32, tag="psa")
                    for kt in range(KT):
                        nc.tensor.matmul(ps[:, 0:256],
                                         w_sb[:, kt, m * P:(m + 1) * P],
                                         hch[:, kt, :],
                                         start=(kt == 0), stop=(kt == KT - 1))
                    if out_sb is not None:
                        nc.vector.tensor_scalar_mul(
                            out_sb[:, m, qc * 256:(qc + 1) * 256], ps[:, 0:256], scale)
                    else:
                        kst = stg.tile([P, 256], bf16, tag="kstage")
                        nc.vector.tensor_copy(kst[:], ps[:, 0:256])
                        nc.sync.dma_start(
                            out_dram[m, :, qc * 256:(qc + 1) * 256], kst[:])

        qk_proj(wq1_d, N_OWN, qT, None, SC)
        nc.sync.dma_start(dbg_q[:], qT[:])
        qk_proj(wk1_d, N, None, kT_dram, None)

        v_sb = big.tile([P, TT_FULL, H, DH + 1], bf16, tag="v33")
        nc.vector.memset(v_sb[:], 1.0)
        w_sb = wres.tile([P, KT, INNER], bf16, tag="wbig")
        nc.sync.dma_start(w_sb[:], wv1_d[:])
        for tt in range(TT_FULL):
            hch = wmed.tile([P, KT, P], bf16, tag="med4")
            nc.sync.dma_start(hch[:], h1T_dram[:, :, tt * P:(tt + 1) * P])
            for nc2 in range(2):
                ps = ps_a.tile([P, 512], f32, tag="psa")
                for kt in range(KT):
                    nc.tensor.matmul(ps[:], hch[:, kt, :],
                                     w_sb[:, kt, nc2 * 512:(nc2 + 1) * 512],
                                     start=(kt == 0), stop=(kt == KT - 1))
                nc.vector.tensor_copy(
                    v_sb[:, tt, nc2 * 8:(nc2 + 1) * 8, 0:DH],
                    ps[:].rearrange("p (hh r) -> p hh r", r=DH))

        # ---------------- attention (shared for self / cross) ----------------
        def attention(get_k, v_t, qT_t, n_keys_tt, out_T):
            for h in range(H):
                hp = (h % 2) * 64
                m2 = h // 2
                kap = get_k(h)  # (P, n_keys) tile; head at partitions hp:hp+64
                for qc in range(2):
                    ex = expp.tile([P, n_keys_tt, 512], bf16, tag="expT")
                    for kt2 in range(max(1, n_keys_tt // 2)):
                        ps_s = ps_sc.tile([P, 1024], f32, tag="pssc")
                        for u in range(min(2, n_keys_tt)):
                            kt = kt2 * 2 + u
                            nc.tensor.matmul(
                                ps_s[:, u * 512:(u + 1) * 512],
                                kap[hp:hp + 64, kt * P:(kt + 1) * P],
                                qT_t[hp:hp + 64, m2, qc * 512:(qc + 1) * 512],
                                start=True, stop=True)
                        nkk = min(2, n_keys_tt)
                        nc.scalar.activation(
                            ex[:, kt2 * 2:kt2 * 2 + nkk, :].rearrange("p a b -> p (a b)"),
                            ps_s[:, 0:nkk * 512], AF.Exp)
                    pavt = ps_av.tile([P, 512], f32, tag="psav")
                    for kt in range(n_keys_tt):
                        fl = dict(start=(kt == 0), stop=(kt == n_keys_tt - 1))
                        if hp == 0:
                            nc.tensor.matmul(pavt[0:DH + 1], v_t[:, kt, h, :],
                                             ex[:, kt, :], **fl)
                        else:
                            nc.tensor.matmul(pavt[64:P], v_t[:, kt, h, 0:DH],
                                             ex[:, kt, :], **fl)
                            nc.tensor.matmul(pavt[0:1], v_t[:, kt, h, DH:DH + 1],
                                             ex[:, kt, :], **fl)
                    sumrow = pavt[DH:DH + 1] if hp == 0 else pavt[0:1]
                    rec = stats.tile([1, 512], bf16, tag="rec")
                    with nc.allow_low_precision(reason="softmax denom bcast"):
                        nc.vector.reciprocal(rec[:], sumrow[:])
                    pbc = ps_a.tile([P, 512], f32, tag="psa")
                    nc.tensor.matmul(pbc[hp:hp + 64, :], ones64[:], rec[:],
                                     start=True, stop=True)
                    bcs = stats.tile([P, 512], f32, tag="bcs")
                    nc.vector.tensor_copy(bcs[hp:hp + 64, :], pbc[hp:hp + 64, :])
                    nc.vector.tensor_tensor(
                        out_T[hp:hp + 64, m2, qc * 512:(qc + 1) * 512],
                        pavt[hp:hp + 64, :], bcs[hp:hp + 64, :], ALU.mult)

        # ---------------- Phase 3: self-attention ----------------
        attn1T = big.tile([P, KT, N_OWN], bf16, tag="attnT")
        _kcache = {}

        def get_k_self(h):
            m2 = h // 2
            if m2 not in _kcache:
                kth = kthp.tile([P, N], bf16, tag="kTh", name="kth")
                nc.sync.dma_start(kth[:], kT_dram[m2])
                _kcache.clear()
                _kcache[m2] = kth
            return _kcache[m2]

        attention(get_k_self, v_sb, qT, TT_FULL, attn1T)
        nc.sync.dma_start(dbg_a1[:], attn1T[:])

        # ---------------- o-proj + residual (generic) ----------------
        def out_proj(attn_T, w_dram, bias_idx, init_src, out_dram):
            # out = o_proj(attn) + bias + residual, written per column chunk
            for dc4 in range(4):
                w_t = wmed.tile([P, KT, 256], bf16, tag="med4")
                nc.sync.dma_start(w_t[:], w_dram[:, :, dc4 * 256:(dc4 + 1) * 256])
                bt = xpool.tile([P, 256], f32, tag="x")
                nc.sync.dma_start(bt[:], bias3_d[:, bias_idx, dc4 * 256:(dc4 + 1) * 256])
                for tt in range(TT_OWN):
                    ps = ps_a.tile([P, 512], f32, tag="psa")
                    for m in range(KT):
                        nc.tensor.matmul(ps[:, 0:256],
                                         attn_T[:, m, tt * P:(tt + 1) * P],
                                         w_t[:, m, :],
                                         start=(m == 0), stop=(m == KT - 1))
                    rt = xpool.tile([P, 256], f32, tag="x")
                    nc.sync.dma_start(rt[:], init_src[tt, :, dc4 * 256:(dc4 + 1) * 256])
                    tmp = evict.tile([P, 256], f32, tag="xn")
                    nc.vector.tensor_tensor(tmp[:], ps[:, 0:256], bt[:], ALU.add)
                    tmp2 = evict.tile([P, 256], f32, tag="xn2")
                    nc.vector.tensor_tensor(tmp2[:], tmp[:], rt[:], ALU.add)
                    nc.sync.dma_start(out_dram[tt, :, dc4 * 256:(dc4 + 1) * 256],
                                      tmp2[:])

        out_proj(attn1T, wo1_d, 0, xf_d, x1_dram)

        # ---------------- Phase 5: LN2 -> h2T; q2 ----------------
        h2T = expp.tile([P, KT, N_OWN], bf16, tag="expT")
        for tt in range(TT_OWN):
            xt = xpool.tile([P, D], f32, tag="x")
            nc.sync.dma_start(xt[:], x1_dram[tt])
            layernorm_tile(xt, tt, 1, dst_sb=h2T)

        q2T = big.tile([P, KT, N_OWN], bf16, tag="qT")
        w_sb = wres.tile([P, KT, INNER], bf16, tag="wbig")
        nc.sync.dma_start(w_sb[:], wq2_d[:])
        for m in range(KT):
            for qc in range(2):
                ps = ps_a.tile([P, 512], f32, tag="psa")
                for kt in range(KT):
                    nc.tensor.matmul(ps[:], w_sb[:, kt, m * P:(m + 1) * P],
                                     h2T[:, kt, qc * 512:(qc + 1) * 512],
                                     start=(kt == 0), stop=(kt == KT - 1))
                nc.vector.tensor_scalar_mul(q2T[:, m, qc * 512:(qc + 1) * 512],
                                            ps[:], SC)

        # ---------------- Phase 6: cross-attention ----------------
        k2T = smk.tile([P, KT, J], bf16, tag="k2T")
        w_sb = wres.tile([P, CKT, INNER], bf16, tag="wbig")
        nc.sync.dma_start(w_sb[:], wk2_d[:])
        for m in range(KT):
            ps = ps_a.tile([P, 512], f32, tag="psa")
            for kt in range(CKT):
                nc.tensor.matmul(ps[:, 0:J], w_sb[:, kt, m * P:(m + 1) * P],
                                 ctxT_sb[:, kt, :],
                                 start=(kt == 0), stop=(kt == CKT - 1))
            nc.vector.tensor_copy(k2T[:, m, :], ps[:, 0:J])
        v2_sb = smk.tile([P, J // P, H, DH + 1], bf16, tag="v2")
        nc.vector.memset(v2_sb[:], 1.0)
        w_sb = wres.tile([P, CKT, INNER], bf16, tag="wbig")
        nc.sync.dma_start(w_sb[:], wv2_d[:])
        for tt in range(J // P):
            for nc2 in range(2):
                ps = ps_a.tile([P, 512], f32, tag="psa")
                for kt in range(CKT):
                    nc.tensor.matmul(ps[:], ctxT_sb[:, kt, tt * P:(tt + 1) * P],
                                     w_sb[:, kt, nc2 * 512:(nc2 + 1) * 512],
                                     start=(kt == 0), stop=(kt == CKT - 1))
                nc.vector.tensor_copy(
                    v2_sb[:, tt, nc2 * 8:(nc2 + 1) * 8, 0:DH],
                    ps[:].rearrange("p (hh r) -> p hh r", r=DH))

        attn2T = big.tile([P, KT, N_OWN], bf16, tag="attnT")

        def get_k_cross(h):
            return k2T[:, h // 2, :]

        attention(get_k_cross, v2_sb, q2T, J // P, attn2T)

        out_proj(attn2T, wo2_d, 1, x1_dram, x2_dram)

        # ---------------- Phase 8: LN3 -> h3T ----------------
        h3T = big.tile([P, KT, N_OWN], bf16, tag="qT")
        for tt in range(TT_OWN):
            xt = xpool.tile([P, D], f32, tag="x")
            nc.sync.dma_start(xt[:], x2_dram[tt])
            layernorm_tile(xt, tt, 2, dst_sb=h3T)

        # ---------------- Phase 9: GEGLU FF ----------------
        g_sb = big.tile([P, 8, N_OWN], bf16, tag="attnT")
        y_sb = big.tile([P, TT_OWN, D], f32, tag="v33")
        for grp in range(4):
            wf2g = wres.tile([P, 8, D], bf16, tag="wbig")
            nc.sync.dma_start(wf2g[:], wf2_d[:, grp * 8:(grp + 1) * 8, :])
            for j in range(8):
                f = grp * 8 + j
                wa = wsm.tile([P, KT, P], bf16, tag="wstream")
                nc.sync.dma_start(wa[:], wf1_d[:, :, f * P:(f + 1) * P])
                wg = wsm.tile([P, KT, P], bf16, tag="wstream")
                nc.sync.dma_start(wg[:], wf1_d[:, :, FF + f * P:FF + (f + 1) * P])
                a_sb = evict.tile([P, N_OWN], bf16, tag="a_sb")
                gt_sb = evict.tile([P, N_OWN], bf16, tag="gt_sb")
                for qc in range(2):
                    sl = slice(qc * 512, (qc + 1) * 512)
                    ps1 = ps_sc.tile([P, 1024], f32, tag="pssc")
                    for kt in range(KT):
                        nc.tensor.matmul(ps1[:, 0:512], wa[:, kt, :], h3T[:, kt, sl],
                                         start=(kt == 0), stop=(kt == KT - 1))
                    nc.vector.tensor_scalar(a_sb[:, sl], ps1[:, 0:512],
                                            fb1_sb[:, f:f + 1], None, ALU.add)
                    ps2 = ps_sc.tile([P, 1024], f32, tag="pssc")
                    for kt in range(KT):
                        nc.tensor.matmul(ps2[:, 0:512], wg[:, kt, :], h3T[:, kt, sl],
                                         start=(kt == 0), stop=(kt == KT - 1))
                    nc.scalar.activation(gt_sb[:, sl], ps2[:, 0:512], AF.Gelu,
                                         bias=fb1_sb[:, 32 + f:32 + f + 1])
                nc.vector.tensor_tensor(g_sb[:, j, :], a_sb[:], gt_sb[:], ALU.mult)
            for tt in range(TT_OWN):
                for dc in range(2):
                    ps = ps_a.tile([P, 512], f32, tag="psa")
                    for j in range(8):
                        nc.tensor.matmul(ps[:], g_sb[:, j, tt * P:(tt + 1) * P],
                                         wf2g[:, j, dc * 512:(dc + 1) * 512],
                                         start=(j == 0), stop=(j == 7))
                    if grp == 0:
                        nc.vector.tensor_copy(
                            y_sb[:, tt, dc * 512:(dc + 1) * 512], ps[:])
                    else:
                        nc.vector.tensor_tensor(
                            y_sb[:, tt, dc * 512:(dc + 1) * 512],
                            y_sb[:, tt, dc * 512:(dc + 1) * 512], ps[:], ALU.add)
        # final: y = y_acc + ff_b2 + x2
        b2t = xpool.tile([P, D], f32, tag="x")
        nc.sync.dma_start(b2t[:], bias3_d[:, 2, :])
        for tt in range(TT_OWN):
            x2t = xpool.tile([P, D], f32, tag="x")
            nc.sync.dma_start(x2t[:], x2_dram[tt])
            yt = evict.tile([P, D], f32, tag="yt")
            nc.vector.tensor_tensor(yt[:], y_sb[:, tt, :], b2t[:], ALU.add)
            nc.vector.tensor_tensor(yt[:], yt[:], x2t[:], ALU.add)
            nc.sync.dma_start(y_d[tt], yt[:])

    nc.compile()
    return nc


def _rearr_w(w, kt):
    return np.ascontiguousarray(
        w.reshape(kt, P, -1).transpose(1, 0, 2)).astype(BF16)


def _shard_inputs(inputs):
    f = {k: np.asarray(v, dtype=np.float32) for k, v in inputs.items()}
    shared = {
        "nw": _rearr_w(np.concatenate([f["n1_w"], f["n2_w"], f["n3_w"]], axis=1), KT),
        "nbc": np.ascontiguousarray(
            np.concatenate([f["n1_b"], f["n2_b"], f["n3_b"]])
            .reshape(3, 16, P).transpose(2, 0, 1).reshape(P, 48)),
        "wq1": _rearr_w(f["q1"], KT), "wk1": _rearr_w(f["k1"], KT),
        "wv1": _rearr_w(f["v1"], KT), "wo1": _rearr_w(f["o1_w"], KT),
        "wq2": _rearr_w(f["q2"], KT), "wk2": _rearr_w(f["k2"], CKT),
        "wv2": _rearr_w(f["v2"], CKT), "wo2": _rearr_w(f["o2_w"], KT),
        "bias3": np.ascontiguousarray(np.broadcast_to(
            np.stack([f["o1_b"], f["o2_b"], f["ff_b2"]])[None], (P, 3, D))),
        "fb1c": np.ascontiguousarray(f["ff_b1"].reshape(64, P).T),
        "wf1": _rearr_w(f["ff_w1"], KT),
        "wf2": _rearr_w(f["ff_w2"], FF // P),
    }
    in_maps = []
    for core in range(8):
        b, half = core // 2, core % 2
        own = f["x"][b, half * N_OWN:(half + 1) * N_OWN]
        oth = f["x"][b, (1 - half) * N_OWN:(2 - half) * N_OWN]
        m = dict(shared)
        m["xf"] = np.ascontiguousarray(
            np.concatenate([own, oth]).reshape(TT_FULL, P, D))
        m["tT"] = np.ascontiguousarray(f["t"][b, 0].reshape(KT, P).T).astype(BF16)
        m["ctxT"] = np.ascontiguousarray(
            f["context"][b].T.reshape(CKT, P, J).transpose(1, 0, 2)).astype(BF16)
        in_maps.append(m)
    return in_maps


def kernel(**inputs):
    from concourse.bass_utils import run_bass_kernel_spmd
    if "nc" not in _CACHE:
        _CACHE["nc"] = _build_program()
    nc = _CACHE["nc"]
    in_maps = _shard_inputs(inputs)
    res = run_bass_kernel_spmd(nc, in_maps, core_ids=list(range(8)))
    out = np.empty((B, N, D), dtype=np.float32)
    for core in range(8):
        b, half = core // 2, core % 2
        out[b, half * N_OWN:(half + 1) * N_OWN] = \
            res.results[core]["y"].reshape(N_OWN, D)
    return out



# revision 10
# speedup vs baseline: 1.6244x; 1.6244x over previous
"""Trainium2 Bass kernel for nn_BasicTransformerBlock_35304631173827.

Sharding: 8 cores = 4 samples x 2 sequence halves. Each core computes its
1024-token half of one sample fully locally (self-attention K/V recomputed
over the full 2048-token sample -> zero collectives). bf16 matmuls with
fp32 PSUM accumulation; LayerNorm stats, softmax and residuals in fp32.

v2: all activations SBUF-resident (no DRAM bounce), AdaLN embeddings
precomputed on host, attention with PSUM-direct exp + col-tiled AV/denom
matmuls + fast reciprocal, LN affine on DVE.
"""

import numpy as np
import ml_dtypes

BF16 = ml_dtypes.bfloat16

B, N, D = 4, 2048, 1024
J, CD = 256, 768
H, DH = 16, 64
INNER = 1024
FF = 4096
P = 128
KT = D // P            # 8
CKT = CD // P          # 6
TT_FULL = N // P       # 16
N_OWN = N // 2
TT_OWN = N_OWN // P    # 8
M2 = H // 2            # 8 head pairs
EPS = 1e-5
SC = DH ** -0.5

_CACHE = {}


def _build_program():
    import concourse.tile as tile
    from concourse import mybir, bacc
    from concourse.masks import make_identity
    from contextlib import ExitStack

    f32 = mybir.dt.float32
    bf16 = mybir.dt.bfloat16
    AF = mybir.ActivationFunctionType
    ALU = mybir.AluOpType

    nc = bacc.Bacc(None, target_bir_lowering=False)

    xf_d = nc.dram_tensor("xf", [TT_FULL, P, D], f32, kind="ExternalInput")
    cols_d = nc.dram_tensor("cols", [P, 48], f32, kind="ExternalInput")
    ctxT_d = nc.dram_tensor("ctxT", [P, CKT, J], bf16, kind="ExternalInput")
    wq1_d = nc.dram_tensor("wq1", [P, KT, INNER], bf16, kind="ExternalInput")
    wk1_d = nc.dram_tensor("wk1", [P, KT, INNER], bf16, kind="ExternalInput")
    wv1_d = nc.dram_tensor("wv1", [P, KT, INNER], bf16, kind="ExternalInput")
    wo1_d = nc.dram_tensor("wo1", [P, KT, D], bf16, kind="ExternalInput")
    wq2_d = nc.dram_tensor("wq2", [P, KT, INNER], bf16, kind="ExternalInput")
    wk2_d = nc.dram_tensor("wk2", [P, CKT, INNER], bf16, kind="ExternalInput")
    wv2_d = nc.dram_tensor("wv2", [P, CKT, INNER], bf16, kind="ExternalInput")
    wo2_d = nc.dram_tensor("wo2", [P, KT, D], bf16, kind="ExternalInput")
    bias3_d = nc.dram_tensor("bias3", [P, 3, D], f32, kind="ExternalInput")
    fb1_d = nc.dram_tensor("fb1c", [P, 64], f32, kind="ExternalInput")
    wf1_d = nc.dram_tensor("wf1", [P, KT, 2 * FF], bf16, kind="ExternalInput")
    wf2_d = nc.dram_tensor("wf2", [P, FF // P, D], bf16, kind="ExternalInput")
    y_d = nc.dram_tensor("y", [TT_OWN, P, D], f32, kind="ExternalOutput")

    with tile.TileContext(nc) as tc, ExitStack() as es:
        konst = es.enter_context(tc.tile_pool(name="konst", bufs=1))
        slotA = es.enter_context(tc.tile_pool(name="slotA", bufs=1))
        slotB = es.enter_context(tc.tile_pool(name="slotB", bufs=1))
        slotC = es.enter_context(tc.tile_pool(name="slotC", bufs=1))
        slotD = es.enter_context(tc.tile_pool(name="slotD", bufs=1))
        slotE = es.enter_context(tc.tile_pool(name="slotE", bufs=1))
        wres = es.enter_context(tc.tile_pool(name="wres", bufs=2))
        wsm = es.enter_context(tc.tile_pool(name="wsm", bufs=2))
        xpool = es.enter_context(tc.tile_pool(name="xpool", bufs=2))
        biasp = es.enter_context(tc.tile_pool(name="biasp", bufs=1))
        evict = es.enter_context(tc.tile_pool(name="evict", bufs=2))
        expool = es.enter_context(tc.tile_pool(name="expool", bufs=2))
        bcsp = es.enter_context(tc.tile_pool(name="bcsp", bufs=2))
        gtp = es.enter_context(tc.tile_pool(name="gtp", bufs=2))
        stats = es.enter_context(tc.tile_pool(name="stats", bufs=2))
        ps_sc = es.enter_context(tc.tile_pool(name="ps_sc", bufs=2, space="PSUM"))
        ps_acc = es.enter_context(tc.tile_pool(name="ps_acc", bufs=2, space="PSUM"))

        # ---------------- constants ----------------
        ident = konst.tile([P, P], bf16)
        make_identity(nc, ident)
        ones64 = konst.tile([P, 64], bf16)
        nc.vector.memset(ones64[:], 1.0)
        eps_t = konst.tile([P, 1], f32)
        nc.vector.memset(eps_t[:], EPS)
        cols = konst.tile([P, 48], f32)
        nc.sync.dma_start(cols[:], cols_d[:])
        ctxT_sb = konst.tile([P, CKT, J], bf16)
        nc.sync.dma_start(ctxT_sb[:], ctxT_d[:])
        fb1_sb = konst.tile([P, 64], f32)
        nc.sync.dma_start(fb1_sb[:], fb1_d[:])

        # big SBUF slots (reused across phases via same tag)
        h1T = slotA.tile([P, KT, N], bf16, tag="A")          # 32KB
        kT_sb = slotB.tile([P, M2, N], bf16, tag="B")        # 32KB
        v_sb = slotC.tile([P, TT_FULL, H, DH], bf16, tag="C")  # 32KB
        qT = slotD.tile([P, KT, N_OWN], bf16, tag="D")       # 16KB
        attn1T = slotE.tile([P, KT, N_OWN], bf16, tag="E")   # 16KB

        # prefetch first weights
        wq1_sb = wres.tile([P, KT, INNER], bf16, tag="w")
        nc.sync.dma_start(wq1_sb[:], wq1_d[:])
        wk1_sb = wres.tile([P, KT, INNER], bf16, tag="w")
        nc.sync.dma_start(wk1_sb[:], wk1_d[:])

        # ---------------- LayerNorm tile ----------------
        def layernorm_tile(x_ap, tt, n3, dst, dst_off):
            """LN + AdaLN affine on (P, D) tile -> transposed chunks into
            dst[:, c, dst_off:dst_off+P]. x_ap: (P, D) f32 (SBUF)."""
            bst = stats.tile([P, 2, 6], f32, tag="bnst")
            for g in range(2):
                nc.vector.bn_stats(bst[:, g, :], x_ap[:, g * 512:(g + 1) * 512])
            mv = stats.tile([P, 6], f32, tag="mv")
            nc.vector.bn_aggr(mv[:, 0:2], bst[:])
            # rs = 1/sqrt(var+eps) = exp(-0.5*ln(var+eps)); Ln+Exp share a
            # table set with attention's Exp (avoids ACT table switches)
            nc.scalar.activation(mv[:, 2:3], mv[:, 1:2], AF.Ln, bias=eps_t[:])
            nc.scalar.activation(mv[:, 3:4], mv[:, 2:3], AF.Exp, scale=-0.5)
            nc.vector.tensor_scalar(mv[:, 4:5], mv[:, 0:1], mv[:, 3:4], -1.0,
                                    ALU.mult, ALU.mult)
            xn = evict.tile([P, D], bf16, tag="xn")
            nc.vector.tensor_scalar(xn[:], x_ap, mv[:, 3:4], mv[:, 4:5],
                                    ALU.mult, ALU.add)
            for c in range(KT):
                pt = ps_acc.tile([P, P], bf16, tag=("avA" if c % 2 == 0 else "avB"),
                                 bufs=1)
                nc.tensor.transpose(pt[:], xn[:, c * P:(c + 1) * P], ident[:])
                nc.vector.tensor_scalar(
                    dst[:, c, dst_off:dst_off + P], pt[:],
                    cols[:, n3 * 16 + c:n3 * 16 + c + 1],
                    cols[:, n3 * 16 + 8 + c:n3 * 16 + 8 + c + 1],
                    ALU.mult, ALU.add)

        # ---------------- projection helpers ----------------
        def proj_chunk(w_sb, m, src, src_sl, n_out, nkt=KT):
            """psum (P, n_out) = W[:, m-chunk]^T @ src[:, :, src_sl]"""
            ps = ps_sc.tile([P, 512], f32, tag="sc")
            for kt in range(nkt):
                nc.tensor.matmul(ps[:, 0:n_out], w_sb[:, kt, m * P:(m + 1) * P],
                                 src[:, kt, src_sl],
                                 start=(kt == 0), stop=(kt == nkt - 1))
            return ps

        # ---------------- Phase 1: LN1 + q1/k1/v1 (interleaved) ------------
        wv1_sb = None
        x_sb = [None] * TT_FULL

        def ln1_tiles(lo, hi):
            for tt in range(lo, hi):
                xt = xpool.tile([P, D], f32, tag="x")
                nc.sync.dma_start(xt[:], xf_d[tt])
                layernorm_tile(xt[:], tt, 0, h1T, tt * P)

        def q1_block(qc):
            for m in range(KT):
                ps = proj_chunk(wq1_sb, m, h1T, slice(qc * 512, (qc + 1) * 512), 512)
                nc.vector.tensor_copy(qT[:, m, qc * 512:(qc + 1) * 512], ps[:])

        def k1_block(c):
            for m2 in range(M2):
                ps = proj_chunk(wk1_sb, m2, h1T, slice(c * 512, (c + 1) * 512), 512)
                nc.vector.tensor_copy(kT_sb[:, m2, c * 512:(c + 1) * 512], ps[:])

        def v1_block(t):
            for nc2 in range(2):
                ps = ps_sc.tile([P, 512], f32, tag="sc")
                for kt in range(KT):
                    nc.tensor.matmul(ps[:], h1T[:, kt, t * P:(t + 1) * P],
                                     wv1_sb[:, kt, nc2 * 512:(nc2 + 1) * 512],
                                     start=(kt == 0), stop=(kt == KT - 1))
                nc.vector.tensor_copy(
                    v_sb[:, t, nc2 * 8:(nc2 + 1) * 8, :],
                    ps[:].rearrange("p (hh r) -> p hh r", r=DH))

        ln1_tiles(0, 4)
        q1_block(0)
        k1_block(0)
        ln1_tiles(4, 8)
        q1_block(1)
        wv1_sb = wres.tile([P, KT, INNER], bf16, tag="w")
        nc.sync.dma_start(wv1_sb[:], wv1_d[:])
        k1_block(1)
        ln1_tiles(8, 12)
        v1_block(0); v1_block(1); v1_block(2); v1_block(3)
        k1_block(2)
        ln1_tiles(12, 16)
        for t in range(4, 8):
            v1_block(t)
        k1_block(3)
        for t in range(8, 16):
            v1_block(t)
        # prefetch o1 weights + bias during attention
        wo1_sb = wres.tile([P, KT, D], bf16, tag="w")
        nc.sync.dma_start(wo1_sb[:], wo1_d[:])
        bias_sb = biasp.tile([P, D], f32, tag="bias")
        nc.sync.dma_start(bias_sb[:], bias3_d[:, 0, :])

        # ---------------- attention (shared self/cross) ----------------
        def attention(n_kt, get_kT, v_t, qT_t, out_T):
            CH = max(1, n_kt // 2)
            for m2 in range(M2):
                for qc in range(2):
                    qsl = slice(qc * 512, (qc + 1) * 512)
                    avh = [ps_acc.tile([P, 512], f32, tag="avA", bufs=1, name="avA"),
                           ps_acc.tile([P, 512], f32, tag="avB", bufs=1, name="avB")]
                    dnh = [ps_acc.tile([P, 512], f32, tag="dnA", bufs=1, name="dnA"),
                           ps_acc.tile([P, 512], f32, tag="dnB", bufs=1, name="dnB")]
                    exs = [[None] * CH, [None] * CH]
                    for c in range(CH + 1):
                        if c < CH:
                            for s in (0, 1):
                                hp = s * 64
                                ps = ps_sc.tile([P, 1024], f32, tag="sc")
                                for u in range(2):
                                    kt = 2 * c + u
                                    nc.tensor.matmul(
                                        ps[:, u * 512:(u + 1) * 512],
                                        get_kT(m2)[hp:hp + 64, kt * P:(kt + 1) * P],
                                        qT_t[hp:hp + 64, m2, qsl],
                                        start=True, stop=True)
                                ex = expool.tile([P, 1024], bf16, tag=f"ex{s}")
                                nc.scalar.activation(ex[:], ps[:], AF.Exp)
                                exs[s][c] = ex
                        if c >= 1:
                            cc = c - 1
                            for u in range(2):
                                kt = 2 * cc + u
                                fl = dict(start=(kt == 0), stop=(kt == n_kt - 1))
                                for s in (0, 1):
                                    h = 2 * m2 + s
                                    exap = exs[s][cc][:, u * 512:(u + 1) * 512]
                                    rsl = slice(s * 64, (s + 1) * 64)
                                    nc.tensor.matmul(
                                        avh[s][rsl, :],
                                        v_t[:, kt, h, :], exap, **fl)
                                    nc.tensor.matmul(
                                        dnh[s][rsl, :],
                                        ones64[:, 0:64], exap, **fl)
                    bcs = bcsp.tile([P, 512], f32, tag="bcs")
                    for s in (0, 1):
                        rsl = slice(s * 64, (s + 1) * 64)
                        nc.vector.reciprocal_approx_fast(out=bcs[rsl, :],
                                                         in_=dnh[s][rsl, :])
                        nc.vector.tensor_tensor(out_T[rsl, m2, qsl],
                                                avh[s][rsl, :], bcs[rsl, :],
                                                ALU.mult)

        def get_k_self(m2):
            return kT_sb[:, m2, :]

        attention(TT_FULL, get_k_self, v_sb, qT, attn1T)

        # ---------------- o-proj + bias + residual ----------------
        def out_proj(attn_T, w_sb, res_get, out_sb):
            # out_sb[:, tt, :] = attn_T.T @ Wo + bias + residual(tt)
            for tt in range(TT_OWN):
                rt = res_get(tt)
                for dc in range(2):
                    dsl = slice(dc * 512, (dc + 1) * 512)
                    ps = ps_sc.tile([P, 512], f32, tag="sc")
                    for m in range(KT):
                        nc.tensor.matmul(ps[:], attn_T[:, m, tt * P:(tt + 1) * P],
                                         w_sb[:, m, dsl],
                                         start=(m == 0), stop=(m == KT - 1))
                    tmp = bcsp.tile([P, 512], f32, tag="bcs")
                    nc.vector.tensor_tensor(tmp[:], ps[:], bias_sb[:, dsl], ALU.add)
                    nc.vector.tensor_tensor(out_sb[:, tt, dsl], tmp[:], rt[:, dsl],
                                            ALU.add)

        x1_sb = slotB.tile([P, TT_OWN, D], f32, tag="B")

        def res1_get(tt):
            xt = xpool.tile([P, D], f32, tag="x")
            nc.sync.dma_start(xt[:], xf_d[tt])
            return xt

        out_proj(attn1T, wo1_sb, res1_get, x1_sb)

        # prefetch q2 weights
        wq2_sb = wres.tile([P, KT, INNER], bf16, tag="w")
        nc.sync.dma_start(wq2_sb[:], wq2_d[:])

        # ---------------- LN2 -> h2T; q2; k2/v2 ----------------
        h2T = slotA.tile([P, KT, N_OWN], bf16, tag="A")
        for tt in range(TT_OWN):
            layernorm_tile(x1_sb[:, tt, :], tt, 1, h2T, tt * P)

        q2T = slotD.tile([P, KT, N_OWN], bf16, tag="D")
        for m in range(KT):
            for qc in range(2):
                ps = proj_chunk(wq2_sb, m, h2T, slice(qc * 512, (qc + 1) * 512), 512)
                nc.vector.tensor_copy(q2T[:, m, qc * 512:(qc + 1) * 512], ps[:])

        wk2_sb = wres.tile([P, CKT, INNER], bf16, tag="w")
        nc.sync.dma_start(wk2_sb[:], wk2_d[:])
        kv2 = slotC.tile([P, 4096], bf16, tag="C")  # k2T (8*256) | v2 (2*16*64)
        k2T = kv2[:, 0:M2 * J].rearrange("p (m j) -> p m j", j=J)
        v2_sb = kv2[:, M2 * J:M2 * J + 2 * H * DH].rearrange(
            "p (t h r) -> p t h r", h=H, r=DH)
        for m2 in range(M2):
            ps = proj_chunk(wk2_sb, m2, ctxT_sb, slice(0, J), J, nkt=CKT)
            nc.vector.tensor_copy(k2T[:, m2, :], ps[:, 0:J])
        wv2_sb = wres.tile([P, CKT, INNER], bf16, tag="w")
        nc.sync.dma_start(wv2_sb[:], wv2_d[:])
        for t in range(2):
            for nc2 in range(2):
                ps = ps_sc.tile([P, 512], f32, tag="sc")
                for kt in range(CKT):
                    nc.tensor.matmul(ps[:], ctxT_sb[:, kt, t * P:(t + 1) * P],
                                     wv2_sb[:, kt, nc2 * 512:(nc2 + 1) * 512],
                                     start=(kt == 0), stop=(kt == CKT - 1))
                nc.vector.tensor_copy(
                    v2_sb[:, t, nc2 * 8:(nc2 + 1) * 8, :],
                    ps[:].rearrange("p (hh r) -> p hh r", r=DH))
        wo2_sb = wres.tile([P, KT, D], bf16, tag="w")
        nc.sync.dma_start(wo2_sb[:], wo2_d[:])

        # ---------------- cross-attention ----------------
        attn2T = slotE.tile([P, KT, N_OWN], bf16, tag="E")

        def get_k_cross(m2):
            return k2T[:, m2, :]

        attention(2, get_k_cross, v2_sb, q2T, attn2T)

        # o2: bias swap + residual from x1_sb
        bias_sb2 = biasp.tile([P, D], f32, tag="bias")
        nc.sync.dma_start(bias_sb2[:], bias3_d[:, 1, :])
        bias_sb = bias_sb2
        x2_sb = slotC.tile([P, TT_OWN, D], f32, tag="C")
        out_proj(attn2T, wo2_sb, lambda tt: x1_sb[:, tt, :], x2_sb)

        # ---------------- LN3 -> h3T ----------------
        h3T = slotA.tile([P, KT, N_OWN], bf16, tag="A")
        for tt in range(TT_OWN):
            layernorm_tile(x2_sb[:, tt, :], tt, 2, h3T, tt * P)

        # ---------------- GEGLU FF ----------------
        y_sb = slotB.tile([P, TT_OWN, D], f32, tag="B")
        g_sb = slotE.tile([P, 8, N_OWN], bf16, tag="E")
        wf2g_sb = wres.tile([P, 8, D], bf16, tag="w")
        nc.sync.dma_start(wf2g_sb[:], wf2_d[:, 0:8, :])
        for grp in range(4):
            wf2g = wf2g_sb
            for j in range(8):
                f = grp * 8 + j
                wa = wsm.tile([P, KT, P], bf16, tag="wa")
                nc.sync.dma_start(wa[:], wf1_d[:, :, f * P:(f + 1) * P])
                wg = wsm.tile([P, KT, P], bf16, tag="wg")
                nc.sync.dma_start(wg[:], wf1_d[:, :, FF + f * P:FF + (f + 1) * P])
                for qc in range(2):
                    sl = slice(qc * 512, (qc + 1) * 512)
                    ps2 = ps_acc.tile([P, 512], f32, tag="avA", bufs=1)
                    for kt in range(KT):
                        nc.tensor.matmul(ps2[:], wg[:, kt, :], h3T[:, kt, sl],
                                         start=(kt == 0), stop=(kt == KT - 1))
                    gt = gtp.tile([P, 512], bf16, tag="gt")
                    nc.scalar.activation(gt[:], ps2[:], AF.Gelu,
                                         bias=fb1_sb[:, 32 + f:32 + f + 1])
                    ps1 = ps_sc.tile([P, 512], f32, tag="sc")
                    for kt in range(KT):
                        nc.tensor.matmul(ps1[:], wa[:, kt, :], h3T[:, kt, sl],
                                         start=(kt == 0), stop=(kt == KT - 1))
                    nc.vector.scalar_tensor_tensor(
                        out=g_sb[:, j, sl], in0=ps1[:], scalar=fb1_sb[:, f:f + 1],
                        in1=gt[:], op0=ALU.add, op1=ALU.mult)
            if grp < 3:
                wf2g_next = wres.tile([P, 8, D], bf16, tag="w")
                nc.sync.dma_start(wf2g_next[:], wf2_d[:, (grp + 1) * 8:(grp + 2) * 8, :])
            for tt in range(TT_OWN):
                for dc in range(2):
                    dsl = slice(dc * 512, (dc + 1) * 512)
                    ps = ps_acc.tile([P, 512], f32, tag="dnA", bufs=1)
                    for j in range(8):
                        nc.tensor.matmul(ps[:], g_sb[:, j, tt * P:(tt + 1) * P],
                                         wf2g[:, j, dsl],
                                         start=(j == 0), stop=(j == 7))
                    if grp == 0:
                        nc.vector.tensor_copy(y_sb[:, tt, dsl], ps[:])
                    else:
                        nc.vector.tensor_tensor(y_sb[:, tt, dsl],
                                                y_sb[:, tt, dsl], ps[:], ALU.add)
            if grp < 3:
                wf2g_sb = wf2g_next

        # final: y = y_acc + ff_b2 + x2
        bias_sb3 = biasp.tile([P, D], f32, tag="bias")
        nc.sync.dma_start(bias_sb3[:], bias3_d[:, 2, :])
        for tt in range(TT_OWN):
            yt = xpool.tile([P, D], f32, tag="x")
            nc.vector.tensor_tensor(yt[:], y_sb[:, tt, :], bias_sb3[:], ALU.add)
            nc.vector.tensor_tensor(yt[:], yt[:], x2_sb[:, tt, :], ALU.add)
            nc.sync.dma_start(y_d[tt], yt[:])

    nc.compile()
    return nc


def _rearr_w(w, kt):
    return np.ascontiguousarray(
        w.reshape(kt, P, -1).transpose(1, 0, 2)).astype(BF16)


def _shard_inputs(inputs):
    f = {k: np.asarray(v, dtype=np.float32) for k, v in inputs.items()}
    nw = np.concatenate([f["n1_w"], f["n2_w"], f["n3_w"]], axis=1)  # (D, 6D)
    nb = np.concatenate([f["n1_b"], f["n2_b"], f["n3_b"]])          # (6D,)
    shared = {
        "wq1": _rearr_w(f["q1"] * SC, KT), "wk1": _rearr_w(f["k1"], KT),
        "wv1": _rearr_w(f["v1"], KT), "wo1": _rearr_w(f["o1_w"], KT),
        "wq2": _rearr_w(f["q2"] * SC, KT), "wk2": _rearr_w(f["k2"], CKT),
        "wv2": _rearr_w(f["v2"], CKT), "wo2": _rearr_w(f["o2_w"], KT),
        "bias3": np.ascontiguousarray(np.broadcast_to(
            np.stack([f["o1_b"], f["o2_b"], f["ff_b2"]])[None], (P, 3, D))),
        "fb1c": np.ascontiguousarray(f["ff_b1"].reshape(64, P).T),
        "wf1": _rearr_w(f["ff_w1"], KT),
        "wf2": _rearr_w(f["ff_w2"], FF // P),
    }
    in_maps = []
    for core in range(8):
        b, half = core // 2, core % 2
        own = f["x"][b, half * N_OWN:(half + 1) * N_OWN]
        oth = f["x"][b, (1 - half) * N_OWN:(2 - half) * N_OWN]
        m = dict(shared)
        m["xf"] = np.ascontiguousarray(
            np.concatenate([own, oth]).reshape(TT_FULL, P, D))
        # AdaLN embeddings on host: emb_n = t @ n_w + n_b -> (scale+1, shift)
        emb = f["t"][b, 0] @ nw + nb                    # (6D,)
        cols = np.empty((P, 48), np.float32)
        for n3 in range(3):
            e = emb[n3 * 2 * D:(n3 + 1) * 2 * D]
            for c in range(KT):
                cols[:, n3 * 16 + c] = e[c * P:(c + 1) * P] + 1.0
                cols[:, n3 * 16 + 8 + c] = e[D + c * P:D + (c + 1) * P]
        m["cols"] = cols
        m["ctxT"] = np.ascontiguousarray(
            f["context"][b].T.reshape(CKT, P, J).transpose(1, 0, 2)).astype(BF16)
        in_maps.append(m)
    return in_maps


def kernel(**inputs):
    from concourse.bass_utils import run_bass_kernel_spmd
    if "nc" not in _CACHE:
        _CACHE["nc"] = _build_program()
    nc = _CACHE["nc"]
    in_maps = _shard_inputs(inputs)
    res = run_bass_kernel_spmd(nc, in_maps, core_ids=list(range(8)))
    out = np.empty((B, N, D), dtype=np.float32)
    for core in range(8):
        b, half = core // 2, core % 2
        out[b, half * N_OWN:(half + 1) * N_OWN] = \
            res.results[core]["y"].reshape(N_OWN, D)
    return out
